# revision 23
# baseline (speedup 1.0000x reference)
"""CommutatorConv2d kernel for Trainium2 (Bass/Tile), 8-core data-parallel.

Math: the reference's commutator/anticommutator conv reduces exactly to a
single-channel 3x3 conv on the channel-summed input:

    out[b] = T @ xs[b] @ A + Bm @ xs[b] @ T + bias,   xs = x.sum(axis=1)

where T is the 128x128 tridiagonal-ones matrix and A, Bm are tridiagonal
matrices built from K's column/row sums scaled by (lambda_c +/- lambda_a):
sum_{i,m} XK[...,i,m] = sum_{i,j} patch[i,j]*colsum(K)[j] and
sum_{j,i} KX[...,j,i] = sum_{m,i} patch[m,i]*rowsum(K)[m], so the effective
3x3 kernel is W[i,j] = a[j] + b[i], separable into a row-conv on the vertical
boxsum plus a col-conv on the horizontal boxsum = the two matrix sandwiches.

Layout: each core's batch shard is handed to the device as [H, B_loc, C, W]
(h-major) so every SBUF partition receives one long contiguous DRAM run per
DMA — 8KB descriptors instead of 512B ones, which is the difference between
~170 GB/s and ~358 GB/s on the HBM path. The device still streams the full
shard HBM->SBUF.

Per core (2 batches x 2 channel-halves): load half -> DVE strided reduce_sum
over c -> accumulate xs_half.T @ [T | Bm.T] into PSUM (the half-combine rides
the matmul accumulation) -> out = uv0.T @ A + uv1.T @ T in PSUM -> bias-add
fused into the PSUM->SBUF copy on DVE -> store. Loads ride the sync HWDGE
ring, stores the scalar ring, tiny constants the gpsimd SWDGE queue.
"""

import numpy as np

B, C, H, W = 16, 32, 128, 128
N_CORES = 8
B_LOC = B // N_CORES
HALF = C // 2

_PROGRAM = None
LAST_RESULTS = None


def _build_program():
    import concourse.mybir as mybir
    from concourse import bacc
    from concourse.bass import MemorySpace
    from concourse.tile import TileContext

    f32 = mybir.dt.float32
    nc = bacc.Bacc(
        "TRN2", target_bir_lowering=False, debug=False, num_devices=N_CORES
    )

    x_dram = nc.dram_tensor("x", (H, B_LOC, C, W), f32, kind="ExternalInput")
    # fused constants: [A | T | TBm | I | bias_col] as columns
    cm_dram = nc.dram_tensor("cmat", (H, 5 * W + 1), f32, kind="ExternalInput")
    # h-major output (host transposes back) -> 1KB contiguous runs per
    # partition and a single store
    out_dram = nc.dram_tensor("out", (H, B_LOC, W), f32, kind="ExternalOutput")

    x_ap = x_dram.ap()
    out_ap = out_dram.ap()

    with TileContext(nc) as tc:
        with (
            tc.tile_pool(name="consts", bufs=1) as cpool,
            tc.tile_pool(name="xpool", bufs=2) as xpool,
            tc.tile_pool(name="uvpool", bufs=2) as uvpool,
            tc.tile_pool(name="opool", bufs=2) as opool,
            tc.tile_pool(name="psum", bufs=2, space=MemorySpace.PSUM) as ppool,
        ):
            # Fused constants on the otherwise-idle scalar HWDGE ring so the
            # identity matrix lands before the first x piece does.
            cm_sb = cpool.tile([H, 5 * W + 1], f32)
            nc.scalar.dma_start(out=cm_sb, in_=cm_dram.ap())
            a_sb = cm_sb[:, 0:W]
            t_sb = cm_sb[:, W : 2 * W]
            tbm_sb = cm_sb[:, 2 * W : 4 * W]
            i_sb = cm_sb[:, 4 * W : 5 * W]
            bias_sb = cm_sb[:, 5 * W : 5 * W + 1]

            # x streams in 8-channel pieces (1024 free elems = 4KB runs per
            # partition, sync HWDGE ring). Per batch: pieces 0-1 fold on the
            # tensor engine (identity-matmul PSUM accumulation), pieces 2-3
            # fold on the vector engine (in-place binary tree). The fold work
            # is split so BOTH engines fit inside the DMA streaming window,
            # and each batch ends on a DVE piece for the shortest tail.
            PIECE = 8  # channels per DMA piece
            PIECES = C // PIECE  # 4
            o2_sb = opool.tile([H, B_LOC * W], f32)
            for b in range(B_LOC):
                # Batch 0 splits its fold between DVE trees (early pieces)
                # and PE identity-quads (late pieces) so both engines work
                # under the DMA window. The LAST batch folds purely on DVE
                # trees — its post-DMA tail chain is the shortest.
                use_pe = b < B_LOC - 1
                tiles = {}
                for p in (2, 3, 0, 1):
                    xq = xpool.tile([H, PIECE * W], f32, tag=f"xq{p}")
                    nc.sync.dma_start(
                        out=xq.rearrange("h (c w) -> h c w", w=W),
                        in_=x_ap[:, b, p * PIECE : (p + 1) * PIECE, :],
                    )
                    tiles[p] = xq

                tree_pieces = (2, 3) if use_pe else (2, 3, 0, 1)
                for p in tree_pieces:
                    xq = tiles[p]
                    n = PIECE * W
                    while n > W:
                        n //= 2
                        nc.vector.tensor_add(xq[:, :n], xq[:, :n], xq[:, n : 2 * n])
                nc.vector.tensor_add(
                    tiles[2][:, :W], tiles[2][:, :W], tiles[3][:, :W]
                )
                xs = tiles[2][:, :W]

                if use_pe:
                    # PE fold of pieces 0-1: cs_psum accumulates four
                    # 4-channel groups elementwise -> [H, 4, W] partials
                    cs_psum = ppool.tile([H, 4 * W], f32)
                    q = 0
                    for p in range(2):
                        for half in range(2):
                            nc.tensor.matmul(
                                cs_psum,
                                i_sb,
                                tiles[p][:, half * 4 * W : (half + 1) * 4 * W],
                                start=(q == 0),
                                stop=(q == 3),
                            )
                            q += 1
                    cs_sb = uvpool.tile([H, 4 * W], f32, tag="cs")
                    nc.vector.tensor_copy(cs_sb, cs_psum)
                    nc.vector.tensor_add(
                        cs_sb[:, : 2 * W],
                        cs_sb[:, : 2 * W],
                        cs_sb[:, 2 * W : 4 * W],
                    )
                    nc.vector.tensor_add(
                        cs_sb[:, :W], cs_sb[:, :W], cs_sb[:, W : 2 * W]
                    )
                    nc.vector.tensor_add(xs, xs, cs_sb[:, :W])
                else:
                    nc.vector.tensor_add(
                        tiles[0][:, :W], tiles[0][:, :W], tiles[1][:, :W]
                    )
                    nc.vector.tensor_add(xs, xs, tiles[0][:, :W])

                uv_psum = ppool.tile([H, 2 * W], f32)
                nc.tensor.matmul(uv_psum, xs, tbm_sb, start=True, stop=True)
                uv_sb = uvpool.tile([H, 2 * W], f32)
                nc.vector.tensor_copy(uv_sb, uv_psum)

                o_psum = ppool.tile([H, W], f32)
                nc.tensor.matmul(o_psum, uv_sb[:, 0:W], a_sb, start=True, stop=False)
                nc.tensor.matmul(
                    o_psum, uv_sb[:, W : 2 * W], t_sb, start=False, stop=True
                )

                # bias-add rides the idle scalar engine, off the DVE queue
                nc.scalar.add(o2_sb[:, b * W : (b + 1) * W], o_psum, add=bias_sb)

            # one store, 1KB runs per partition, on the idle SWDGE path
            nc.gpsimd.dma_start(
                out=out_ap, in_=o2_sb.rearrange("h (b w) -> h b w", w=W)
            )

    nc.compile()
    return nc


def _get_program():
    global _PROGRAM
    if _PROGRAM is None:
        _PROGRAM = _build_program()
    return _PROGRAM


def _build_consts(K, bias, lambda_c, lambda_a):
    K = np.asarray(K, np.float32)
    lc = float(np.asarray(lambda_c))
    la = float(np.asarray(lambda_a))
    a = (lc + la) * K.sum(axis=0)  # column sums -> horizontal taps
    b = (la - lc) * K.sum(axis=1)  # row sums -> vertical taps
    eye = np.eye(H, dtype=np.float32)
    up = np.eye(H, k=1, dtype=np.float32)
    dn = np.eye(H, k=-1, dtype=np.float32)
    T = eye + up + dn
    A = a[1] * eye + a[0] * up + a[2] * dn
    Bm = b[1] * eye + b[2] * up + b[0] * dn
    bias_col = np.full((H, 1), np.asarray(bias, np.float32).reshape(-1)[0], np.float32)
    # fused [A | T | T | Bm.T | I | bias_col] -> [H, 5W+1]
    cm = np.concatenate([A, T, T, Bm.T, eye, bias_col], axis=1)
    return np.ascontiguousarray(cm, np.float32)


def kernel(x, K, bias, lambda_c, lambda_a, _trace=False):
    global LAST_RESULTS
    from concourse.bass_utils import run_bass_kernel_spmd

    x = np.asarray(x, np.float32)
    cm = _build_consts(K, bias, lambda_c, lambda_a)
    nc = _get_program()

    in_maps = []
    for core in range(N_CORES):
        shard = x[core * B_LOC : (core + 1) * B_LOC]  # [B_LOC, C, H, W]
        shard_t = np.ascontiguousarray(shard.transpose(2, 0, 1, 3))  # [H,B,C,W]
        in_maps.append({"x": shard_t, "cmat": cm})

    res = run_bass_kernel_spmd(
        nc, in_maps, core_ids=list(range(N_CORES)), trace=_trace
    )
    LAST_RESULTS = res
    # per-core outputs are [H, B_LOC, W]; swap back to [B_LOC, H, W]
    out = np.concatenate(
        [r["out"].transpose(1, 0, 2) for r in res.results], axis=0
    )
    return out.reshape(B, 1, H, W).astype(np.float32, copy=False)


# revision 25
# speedup vs baseline: 1.0312x; 1.0312x over previous
"""CommutatorConv2d kernel for Trainium2 (Bass/Tile), 8-core data-parallel.

Math: the reference's commutator/anticommutator conv reduces exactly to a
single-channel 3x3 conv on the channel-summed input:

    out[b] = T @ xs[b] @ A + Bm @ xs[b] @ T + bias,   xs = x.sum(axis=1)

where T is the 128x128 tridiagonal-ones matrix and A, Bm are tridiagonal
matrices built from K's column/row sums scaled by (lambda_c +/- lambda_a):
sum_{i,m} XK[...,i,m] = sum_{i,j} patch[i,j]*colsum(K)[j] and
sum_{j,i} KX[...,j,i] = sum_{m,i} patch[m,i]*rowsum(K)[m], so the effective
3x3 kernel is W[i,j] = a[j] + b[i], separable into a row-conv on the vertical
boxsum plus a col-conv on the horizontal boxsum = the two matrix sandwiches.

Layout: each core's batch shard is handed to the device as [H, B_loc, C, W]
(h-major) so every SBUF partition receives one long contiguous DRAM run per
DMA — 8KB descriptors instead of 512B ones, which is the difference between
~170 GB/s and ~358 GB/s on the HBM path. The device still streams the full
shard HBM->SBUF.

Per core (2 batches x 4 pieces of 8 channels): the channel fold is split
between the vector engine (in-place contiguous binary-tree adds over the
early-arriving pieces — hidden under the DMA window) and the tensor engine
(identity-matmul PSUM accumulation over the late pieces — the shorter
post-DMA dependency chain). Then uv = xs.T @ [T | Bm.T] (one K=128,N=256
matmul), out = uv[:, :128].T @ A + uv[:, 128:].T @ T accumulated in PSUM,
bias-add on the scalar engine into a combined output tile, and one 1KB-run
store on the SWDGE path. x pieces ride the sync HWDGE ring; the fused
constant matrix rides the scalar ring so the identity lands first.
"""

import numpy as np

B, C, H, W = 16, 32, 128, 128
N_CORES = 8
B_LOC = B // N_CORES

_PROGRAM = None
LAST_RESULTS = None


def _build_program():
    import concourse.mybir as mybir
    from concourse import bacc
    from concourse.bass import MemorySpace
    from concourse.tile import TileContext

    f32 = mybir.dt.float32
    nc = bacc.Bacc(
        "TRN2", target_bir_lowering=False, debug=False, num_devices=N_CORES
    )

    x_dram = nc.dram_tensor("x", (H, B_LOC, C, W), f32, kind="ExternalInput")
    # fused constants: [A | T | TBm | I | bias_col] as columns
    cm_dram = nc.dram_tensor("cmat", (H, 5 * W + 1), f32, kind="ExternalInput")
    # h-major output (host transposes back) -> 1KB contiguous runs per
    # partition and a single store
    out_dram = nc.dram_tensor("out", (H, B_LOC, W), f32, kind="ExternalOutput")

    x_ap = x_dram.ap()
    out_ap = out_dram.ap()

    with TileContext(nc) as tc:
        with (
            tc.tile_pool(name="consts", bufs=1) as cpool,
            tc.tile_pool(name="xpool", bufs=2) as xpool,
            tc.tile_pool(name="uvpool", bufs=2) as uvpool,
            tc.tile_pool(name="opool", bufs=2) as opool,
            tc.tile_pool(name="psum", bufs=2, space=MemorySpace.PSUM) as ppool,
        ):
            # Fused constants on the otherwise-idle scalar HWDGE ring so the
            # identity matrix lands before the first x piece does.
            cm_sb = cpool.tile([H, 5 * W + 1], f32)
            nc.scalar.dma_start(out=cm_sb, in_=cm_dram.ap())
            a_sb = cm_sb[:, 0:W]
            t_sb = cm_sb[:, W : 2 * W]
            tbm_sb = cm_sb[:, 2 * W : 4 * W]
            i_sb = cm_sb[:, 4 * W : 5 * W]
            bias_sb = cm_sb[:, 5 * W : 5 * W + 1]

            # x streams in 8-channel pieces (1024 free elems = 4KB runs per
            # partition, sync HWDGE ring). Per batch: pieces 0-1 fold on the
            # tensor engine (identity-matmul PSUM accumulation), pieces 2-3
            # fold on the vector engine (in-place binary tree). The fold work
            # is split so BOTH engines fit inside the DMA streaming window,
            # and each batch ends on a DVE piece for the shortest tail.
            PIECE = 8  # channels per DMA piece
            PIECES = C // PIECE  # 4
            o2_sb = opool.tile([H, B_LOC * W], f32)
            for b in range(B_LOC):
                # Each batch splits its fold between DVE trees (early pieces,
                # hidden under the DMA window) and PE identity-quads (late
                # pieces — the post-DMA chain through the tensor engine is
                # the shorter one).
                use_pe = True
                tiles = {}
                for p in (2, 3, 0, 1):
                    xq = xpool.tile([H, PIECE * W], f32, tag=f"xq{p}")
                    nc.sync.dma_start(
                        out=xq.rearrange("h (c w) -> h c w", w=W),
                        in_=x_ap[:, b, p * PIECE : (p + 1) * PIECE, :],
                    )
                    tiles[p] = xq

                tree_pieces = (2, 3) if use_pe else (2, 3, 0, 1)
                for p in tree_pieces:
                    xq = tiles[p]
                    n = PIECE * W
                    while n > W:
                        n //= 2
                        nc.vector.tensor_add(xq[:, :n], xq[:, :n], xq[:, n : 2 * n])
                nc.vector.tensor_add(
                    tiles[2][:, :W], tiles[2][:, :W], tiles[3][:, :W]
                )
                xs = tiles[2][:, :W]

                if use_pe:
                    # PE fold of pieces 0-1: cs_psum accumulates four
                    # 4-channel groups elementwise -> [H, 4, W] partials
                    cs_psum = ppool.tile([H, 4 * W], f32)
                    q = 0
                    for p in range(2):
                        for half in range(2):
                            nc.tensor.matmul(
                                cs_psum,
                                i_sb,
                                tiles[p][:, half * 4 * W : (half + 1) * 4 * W],
                                start=(q == 0),
                                stop=(q == 3),
                            )
                            q += 1
                    cs_sb = uvpool.tile([H, 4 * W], f32, tag="cs")
                    nc.vector.tensor_copy(cs_sb, cs_psum)
                    nc.vector.tensor_add(
                        cs_sb[:, : 2 * W],
                        cs_sb[:, : 2 * W],
                        cs_sb[:, 2 * W : 4 * W],
                    )
                    nc.vector.tensor_add(
                        cs_sb[:, :W], cs_sb[:, :W], cs_sb[:, W : 2 * W]
                    )
                    nc.vector.tensor_add(xs, xs, cs_sb[:, :W])
                else:
                    nc.vector.tensor_add(
                        tiles[0][:, :W], tiles[0][:, :W], tiles[1][:, :W]
                    )
                    nc.vector.tensor_add(xs, xs, tiles[0][:, :W])

                uv_psum = ppool.tile([H, 2 * W], f32)
                nc.tensor.matmul(uv_psum, xs, tbm_sb, start=True, stop=True)
                uv_sb = uvpool.tile([H, 2 * W], f32)
                nc.vector.tensor_copy(uv_sb, uv_psum)

                o_psum = ppool.tile([H, W], f32)
                nc.tensor.matmul(o_psum, uv_sb[:, 0:W], a_sb, start=True, stop=False)
                nc.tensor.matmul(
                    o_psum, uv_sb[:, W : 2 * W], t_sb, start=False, stop=True
                )

                # bias-add rides the idle scalar engine, off the DVE queue
                nc.scalar.add(o2_sb[:, b * W : (b + 1) * W], o_psum, add=bias_sb)

            # one store, 1KB runs per partition, on the idle SWDGE path
            nc.gpsimd.dma_start(
                out=out_ap, in_=o2_sb.rearrange("h (b w) -> h b w", w=W)
            )

    nc.compile()
    return nc


def _get_program():
    global _PROGRAM
    if _PROGRAM is None:
        _PROGRAM = _build_program()
    return _PROGRAM


def _build_consts(K, bias, lambda_c, lambda_a):
    K = np.asarray(K, np.float32)
    lc = float(np.asarray(lambda_c))
    la = float(np.asarray(lambda_a))
    a = (lc + la) * K.sum(axis=0)  # column sums -> horizontal taps
    b = (la - lc) * K.sum(axis=1)  # row sums -> vertical taps
    eye = np.eye(H, dtype=np.float32)
    up = np.eye(H, k=1, dtype=np.float32)
    dn = np.eye(H, k=-1, dtype=np.float32)
    T = eye + up + dn
    A = a[1] * eye + a[0] * up + a[2] * dn
    Bm = b[1] * eye + b[2] * up + b[0] * dn
    bias_col = np.full((H, 1), np.asarray(bias, np.float32).reshape(-1)[0], np.float32)
    # fused [A | T | T | Bm.T | I | bias_col] -> [H, 5W+1]
    cm = np.concatenate([A, T, T, Bm.T, eye, bias_col], axis=1)
    return np.ascontiguousarray(cm, np.float32)


def kernel(x, K, bias, lambda_c, lambda_a, _trace=False):
    global LAST_RESULTS
    from concourse.bass_utils import run_bass_kernel_spmd

    x = np.asarray(x, np.float32)
    cm = _build_consts(K, bias, lambda_c, lambda_a)
    nc = _get_program()

    in_maps = []
    for core in range(N_CORES):
        shard = x[core * B_LOC : (core + 1) * B_LOC]  # [B_LOC, C, H, W]
        shard_t = np.ascontiguousarray(shard.transpose(2, 0, 1, 3))  # [H,B,C,W]
        in_maps.append({"x": shard_t, "cmat": cm})

    res = run_bass_kernel_spmd(
        nc, in_maps, core_ids=list(range(N_CORES)), trace=_trace
    )
    LAST_RESULTS = res
    # per-core outputs are [H, B_LOC, W]; swap back to [B_LOC, H, W]
    out = np.concatenate(
        [r["out"].transpose(1, 0, 2) for r in res.results], axis=0
    )
    return out.reshape(B, 1, H, W).astype(np.float32, copy=False)
